# revision 18
# baseline (speedup 1.0000x reference)
"""MoE router (AutonomousRouter) for TRN2, 8 NeuronCores.

Computes reference:
    act    = einsum('bnd,edc->bnec', x, W)          B,N,D,E,C = 4,2048,2048,8,512
    logits = ||act||_2 over c                       [B,N,E]
    probs  = softmax(logits, -1)
    top-2 routing with capacity 640 (priority = order within k-major (choice, token) sequence)
    out    = stack([dispatch, combine])             [2,B,N,E,640] fp32

Sharding: data-parallel over tokens; core i <- tokens [i*1024, (i+1)*1024) of the
flattened [8192] token axis. Weights replicated.

Three device phases:
  A1 (coarse): single fp16 matmul per k-tile -> sum-of-squares ss [TOK, E].
      fp16 logit error is <~2e-3 while decision gaps are almost always larger;
      only tokens whose top1/2/3 logit gaps fall under GAP_T need exactness.
  A2 (exact):  fp16 hi/lo split (3 full-rate matmuls, fp32-grade: ~1e-7 logit
      err, measured on HW) for the <=NP ambiguous tokens, expert e on core e.
  B  (rows):   for each (token, choice) build the combine row
      prob*(iota==slot) densely as fp16 [2*TOK, cap]; host glue scatters rows
      into the zero output during unsharding (no indirect DMA) and derives
      dispatch = (combine != 0), exact since top-2 probs >> fp16 tiny.
Host glue between phases: softmax/top-2/capacity-cumsum on [8192, 8] scalars.
"""
import numpy as np

import concourse.bacc as bacc
import concourse.mybir as mybir
from concourse.tile import TileContext
from concourse.bass_utils import run_bass_kernel_spmd

P = 128          # partitions
B, N, D, E, C = 4, 2048, 2048, 8, 512
CAP = 640
NCORES = 8
T = B * N
TOK = T // NCORES           # tokens per core = 1024
NT = TOK // P               # token tiles per core = 8
KT = D // P                 # contraction tiles = 16

W_SCALE = 32.0              # keep fp16 weights away from subnormals
LO_SCALE = 4096.0           # 2^12 scaling for fp16 split low parts
GAP_T = 9e-3                # coarse logit-gap ambiguity threshold
NP_OPTS = (384, 512)        # padded ambiguous-token capacities (3 or 4 tiles)

f32 = mybir.dt.float32
f16 = mybir.dt.float16

_cache = {}
LAST_IN_MAPS_A1 = None   # kept for test harness re-runs/profiling
LAST_IN_MAPS_A2 = None
LAST_IN_MAPS_B = None
LAST_NP = None
LAST_NAMB = None


def _build_a1():
    """Coarse pass: ss[t, e] = sum_c (x[t] @ (32*w[e]))_c^2 in fp16 x fp16."""
    nc = bacc.Bacc("TRN2", target_bir_lowering=False, debug=False, num_devices=NCORES)
    xT = nc.dram_tensor("xT", [D, TOK], f16, kind="ExternalInput")
    w = nc.dram_tensor("w", [E, D, C], f16, kind="ExternalInput")
    ss_out = nc.dram_tensor("ss", [TOK, E], f32, kind="ExternalOutput")

    with TileContext(nc) as tc:
        with (
            tc.tile_pool(name="const", bufs=1) as cpool,
            tc.tile_pool(name="work", bufs=3) as spool,
            tc.tile_pool(name="psum", bufs=8, space="PSUM") as psum,
        ):
            # x^T and all of W live in SBUF (21 MB fp16). DMAs are issued in
            # consumption order; the first chunks are single k-blocks so the
            # first matmuls wait on ~0.8MB instead of 21MB.
            CHUNKS = [1, 3, 4, 4, 4]           # k-blocks per chunk, sums to KT
            CH0 = [sum(CHUNKS[:i]) for i in range(len(CHUNKS))]
            NCH = len(CHUNKS)

            def _x_chunk(q):
                nk = CHUNKS[q]
                t_ = cpool.tile([P, nk * TOK], f16, tag=f"xq{q}", name=f"x{q}")
                nc.sync.dma_start(
                    out=t_[:].rearrange("p (k n) -> p k n", k=nk),
                    in_=xT.ap()[CH0[q] * P:(CH0[q] + nk) * P, :]
                        .rearrange("(k p) n -> p k n", p=P),
                )
                return t_

            def _w_chunk(e, q):
                nk = CHUNKS[q]
                t_ = cpool.tile([P, nk * C], f16, tag=f"w{e}q{q}", name=f"w{e}_{q}")
                nc.sync.dma_start(
                    out=t_[:].rearrange("p (k c) -> p k c", k=nk),
                    in_=w.ap()[e, CH0[q] * P:(CH0[q] + nk) * P, :]
                        .rearrange("(k p) c -> p k c", p=P),
                )
                return t_

            xq, w_sb = [], {}
            for q in range(NCH):
                w_sb[(0, q)] = _w_chunk(0, q)
                xq.append(_x_chunk(q))
            for e in range(1, E):
                for q in range(NCH):
                    w_sb[(e, q)] = _w_chunk(e, q)

            ss_sb = cpool.tile([P, NT * E], f32, tag="ss")

            for e in range(E):
                for t in range(NT):
                    ps = psum.tile([P, C], f32, space="PSUM", tag="ps")
                    for k in range(KT):
                        q = max(i for i in range(NCH) if CH0[i] <= k)
                        kq = k - CH0[q]
                        nc.tensor.matmul(
                            ps[:],
                            lhsT=xq[q][:, kq * TOK + t * P: kq * TOK + (t + 1) * P],
                            rhs=w_sb[(e, q)][:, kq * C:(kq + 1) * C],
                            start=(k == 0), stop=(k == KT - 1),
                        )
                    sq = spool.tile([P, C], f32, tag="sq")
                    nc.scalar.activation(sq[:], ps[:], mybir.ActivationFunctionType.Square,
                                         accum_out=ss_sb[:, t * E + e: t * E + e + 1])
            nc.sync.dma_start(
                out=ss_out.ap()[:, :].rearrange("(t p) e -> p t e", p=P),
                in_=ss_sb[:].rearrange("p (t e) -> p t e", e=E))
    nc.compile()
    return nc


def _build_a2(npad):
    """Exact pass: fp32-grade sumsq for npad gathered tokens x one expert/core.

    x = xh + xls/LO_SCALE, w = wh + wls/LO_SCALE (all fp16);
    a ~= xh@wh + (xh@wls + xls@wh)/LO_SCALE  (xl*wl term ~2^-22 rel, dropped).
    """
    nc = bacc.Bacc("TRN2", target_bir_lowering=False, debug=False, num_devices=NCORES)
    xh = nc.dram_tensor("xh", [D, npad], f16, kind="ExternalInput")
    xls = nc.dram_tensor("xls", [D, npad], f16, kind="ExternalInput")
    wh = nc.dram_tensor("wh", [D, C], f16, kind="ExternalInput")
    wls = nc.dram_tensor("wls", [D, C], f16, kind="ExternalInput")
    ss_out = nc.dram_tensor("ss", [npad, 1], f32, kind="ExternalOutput")
    NT2 = npad // P

    CHUNKS = [1, 3, 4, 4, 4]     # k-blocks per chunk, consumption order
    CH0 = [sum(CHUNKS[:i]) for i in range(len(CHUNKS))]
    NCH = len(CHUNKS)

    with TileContext(nc) as tc:
        with (
            tc.tile_pool(name="const", bufs=1) as cpool,
            tc.tile_pool(name="work", bufs=3) as spool,
            tc.tile_pool(name="psum", bufs=1, space="PSUM") as psum,
        ):
            def _chunk(dram, cols, q, tag):
                nk = CHUNKS[q]
                t_ = cpool.tile([P, nk * cols], f16, tag=tag, name=tag)
                nc.sync.dma_start(
                    out=t_[:].rearrange("p (k n) -> p k n", k=nk),
                    in_=dram.ap()[CH0[q] * P:(CH0[q] + nk) * P, :]
                        .rearrange("(k p) n -> p k n", p=P),
                )
                return t_

            wh_q, xh_q, wls_q, xls_q = [], [], [], []
            for q in range(NCH):
                wh_q.append(_chunk(wh, C, q, f"wh{q}"))
                xh_q.append(_chunk(xh, npad, q, f"xh{q}"))
                wls_q.append(_chunk(wls, C, q, f"wls{q}"))
                xls_q.append(_chunk(xls, npad, q, f"xls{q}"))
            ss_sb = cpool.tile([P, NT2], f32, tag="ss")

            # k-outer / tile-inner: up to 8 PSUM banks = NT2 tiles x {hh, cross};
            # each DMA chunk is consumed by all tiles before the next chunk.
            # The last chunk runs tile-outer so tiles finish staggered and the
            # combine/Square tail overlaps the matmul stream.
            ps_hh = [psum.tile([P, C], f32, space="PSUM", tag=f"ph{t}",
                               name=f"ph{t}") for t in range(NT2)]
            ps_cr = [psum.tile([P, C], f32, space="PSUM", tag=f"pc{t}",
                               name=f"pc{t}") for t in range(NT2)]

            def _mms(t, k):
                q = max(i for i in range(NCH) if CH0[i] <= k)
                kq = k - CH0[q]
                xh_blk = xh_q[q][:, kq * npad + t * P: kq * npad + (t + 1) * P]
                xls_blk = xls_q[q][:, kq * npad + t * P: kq * npad + (t + 1) * P]
                wh_blk = wh_q[q][:, kq * C:(kq + 1) * C]
                wls_blk = wls_q[q][:, kq * C:(kq + 1) * C]
                nc.tensor.matmul(ps_hh[t][:], lhsT=xh_blk, rhs=wh_blk,
                                 start=(k == 0), stop=(k == KT - 1))
                nc.tensor.matmul(ps_cr[t][:], lhsT=xh_blk, rhs=wls_blk,
                                 start=(k == 0), stop=False)
                nc.tensor.matmul(ps_cr[t][:], lhsT=xls_blk, rhs=wh_blk,
                                 start=False, stop=(k == KT - 1))

            def _finish(t):
                a = spool.tile([P, C], f32, tag="a")
                nc.vector.tensor_scalar(a[:], ps_cr[t][:], 1.0 / LO_SCALE, None,
                                        op0=mybir.AluOpType.mult)
                nc.vector.tensor_add(out=a[:], in0=a[:], in1=ps_hh[t][:])
                sq = spool.tile([P, C], f32, tag="sq")
                nc.scalar.activation(sq[:], a[:], mybir.ActivationFunctionType.Square,
                                     accum_out=ss_sb[:, t:t + 1])

            LASTQ = CH0[-1]                     # k-start of the last chunk
            for k in range(LASTQ):
                for t in range(NT2):
                    _mms(t, k)
            for t in range(NT2):
                for k in range(LASTQ, KT):
                    _mms(t, k)
                _finish(t)
            nc.sync.dma_start(
                out=ss_out.ap()[:, 0].rearrange("(t p) -> p t", p=P), in_=ss_sb[:])
    nc.compile()
    return nc


def _build_b(cap=CAP):
    """Row builder: per (token, choice) the combine row prob*(iota==slot),
    dense fp16 [2*TOK, cap]. Host glue scatters rows by (token, chosen
    expert) while unsharding and derives dispatch = (combine != 0), exact
    because top-2 softmax probs (>~1e-2) never round to fp16 zero."""
    NR = 2 * TOK
    NG = NR // P
    nc = bacc.Bacc("TRN2", target_bir_lowering=False, debug=False, num_devices=NCORES)
    slot = nc.dram_tensor("slot", [NR, 1], f32, kind="ExternalInput")
    prob = nc.dram_tensor("prob", [NR, 1], f32, kind="ExternalInput")
    iota_cap = nc.dram_tensor("iota_cap", [P, cap], f16, kind="ExternalInput")
    rows = nc.dram_tensor("rows", [NR, cap], f16, kind="ExternalOutput")

    with TileContext(nc) as tc:
        with (
            tc.tile_pool(name="const", bufs=1) as cpool,
            tc.tile_pool(name="work", bufs=4) as spool,
        ):
            iota_sb = cpool.tile([P, cap], f16, tag="iota")
            nc.sync.dma_start(out=iota_sb[:], in_=iota_cap.ap()[:, :])
            sl = cpool.tile([P, NG], f32, tag="sl")
            nc.sync.dma_start(out=sl[:], in_=slot.ap()[:, 0].rearrange("(g p) -> p g", p=P))
            pr = cpool.tile([P, NG], f32, tag="pr")
            nc.sync.dma_start(out=pr[:], in_=prob.ap()[:, 0].rearrange("(g p) -> p g", p=P))
            GPB = 4                           # groups batched per DMA
            for g0 in range(0, NG, GPB):
                rtile = spool.tile([P, GPB * cap], f16, tag="rt")
                for j in range(GPB):
                    g = g0 + j
                    nc.vector.tensor_scalar(rtile[:, j * cap:(j + 1) * cap],
                                            iota_sb[:], sl[:, g:g + 1],
                                            pr[:, g:g + 1],
                                            op0=mybir.AluOpType.is_equal,
                                            op1=mybir.AluOpType.mult)
                nc.sync.dma_start(
                    out=rows.ap()[g0 * P:(g0 + GPB) * P, :]
                        .rearrange("(g p) c -> p g c", p=P),
                    in_=rtile[:].rearrange("p (g c) -> p g c", g=GPB))
    nc.compile()
    return nc


def _get(name, builder):
    if name not in _cache:
        _cache[name] = builder()
    return _cache[name]


def kernel(token_inputs, bottleneck_weights, expert_capacity):
    global LAST_IN_MAPS_A1, LAST_IN_MAPS_A2, LAST_IN_MAPS_B, LAST_NP, LAST_NAMB
    x = np.ascontiguousarray(np.asarray(token_inputs, dtype=np.float32)).reshape(T, D)
    w = np.ascontiguousarray(np.asarray(bottleneck_weights, dtype=np.float32))
    cap = int(expert_capacity)
    assert 0 < cap <= 2048   # iota/slot compares rely on exact fp16 integers
    core_ids = list(range(NCORES))

    # ---- phase A1: coarse fp16 sum-of-squares ----
    w16 = np.ascontiguousarray((w * W_SCALE).astype(np.float16))
    in_maps_a1 = []
    for c in core_ids:
        xT = np.ascontiguousarray(x[c * TOK:(c + 1) * TOK].T).astype(np.float16)
        in_maps_a1.append({"xT": xT, "w": w16})
    LAST_IN_MAPS_A1 = in_maps_a1
    nc1 = _get("a1", _build_a1)
    res1 = run_bass_kernel_spmd(nc1, in_maps_a1, core_ids)
    ss = np.concatenate([r["ss"] for r in res1.results], 0).astype(np.float64)
    ss /= W_SCALE * W_SCALE
    L = np.sqrt(ss)                                   # coarse logits [T, E]

    # ---- ambiguous tokens: any of the top1/2/3 coarse gaps under GAP_T ----
    l_sorted = np.sort(L, axis=1)[:, ::-1]
    rel_gap = np.minimum(l_sorted[:, 0] - l_sorted[:, 1],
                         l_sorted[:, 1] - l_sorted[:, 2])
    amb = np.flatnonzero(rel_gap < GAP_T)
    namb = len(amb)
    NP = next((n for n in NP_OPTS if namb <= n), None)
    assert NP is not None, f"ambiguous token overflow: {namb} > {NP_OPTS[-1]}"
    LAST_NP, LAST_NAMB = NP, namb

    # ---- phase A2: exact sumsq for ambiguous tokens (expert e on core e) ----
    xaT = np.zeros((D, NP), np.float32)
    xaT[:, :namb] = x[amb].T
    xh = xaT.astype(np.float16)
    xls = ((xaT - xh.astype(np.float32)) * LO_SCALE).astype(np.float16)
    in_maps_a2 = []
    for e in range(NCORES):
        we = np.ascontiguousarray(w[e])
        wh = we.astype(np.float16)
        wls = ((we - wh.astype(np.float32)) * LO_SCALE).astype(np.float16)
        in_maps_a2.append({"xh": xh, "xls": xls, "wh": wh, "wls": wls})
    LAST_IN_MAPS_A2 = in_maps_a2
    nc2 = _get(f"a2_{NP}", lambda: _build_a2(NP))
    res2 = run_bass_kernel_spmd(nc2, in_maps_a2, core_ids)
    if namb:
        ss_ex = np.stack([res2.results[e]["ss"].reshape(-1)[:namb]
                          for e in range(NCORES)], 1).astype(np.float64)
        L[amb] = np.sqrt(ss_ex)

    # ---- host glue: top-2 (stable => lower index on ties, like lax.top_k),
    # softmax probs, capacity priorities over the k-major (choice, token) seq.
    order = np.argsort(-L, axis=1, kind="stable")
    e0, e1 = order[:, 0], order[:, 1]
    m = L.max(1, keepdims=True)
    pexp = np.exp(L - m)
    probs = pexp / pexp.sum(1, keepdims=True)
    slot = np.empty((T, 2), np.int64)
    for b in range(B):
        bsl = slice(b * N, (b + 1) * N)
        seq = np.concatenate([e0[bsl], e1[bsl]])
        onehot = seq[:, None] == np.arange(E)[None, :]
        pri = onehot.cumsum(0) - 1
        pv = pri[np.arange(2 * N), seq]
        slot[bsl, 0] = pv[:N]
        slot[bsl, 1] = pv[N:]

    # ---- phase B: build rows on device ----
    ar = np.arange(T)
    p0 = probs[ar, e0].astype(np.float32)
    p1 = probs[ar, e1].astype(np.float32)
    iota16 = np.tile(np.arange(cap, dtype=np.float16), (P, 1))
    in_maps_b = []
    for c in core_ids:
        tsl = slice(c * TOK, (c + 1) * TOK)
        in_maps_b.append({
            "slot": np.concatenate([slot[tsl, 0], slot[tsl, 1]])
                      .astype(np.float32)[:, None],
            "prob": np.concatenate([p0[tsl], p1[tsl]]).astype(np.float32)[:, None],
            "iota_cap": iota16,
        })
    LAST_IN_MAPS_B = in_maps_b
    nc3 = _get(f"b{cap}", lambda: _build_b(cap))
    res3 = run_bass_kernel_spmd(nc3, in_maps_b, core_ids)

    # ---- unshard: scatter rows into the dense output ----
    out = np.zeros((2, T, E, cap), np.float32)
    for c in core_ids:
        rows = res3.results[c]["rows"]                  # [2*TOK, cap] f16
        toks = np.arange(c * TOK, (c + 1) * TOK)
        for k, ek in ((0, e0), (1, e1)):
            rk = rows[k * TOK:(k + 1) * TOK].astype(np.float32)
            out[0, toks, ek[toks]] = (rk != 0.0).astype(np.float32)
            out[1, toks, ek[toks]] = rk
    return out.reshape(2, B, N, E, cap)


# revision 19
# speedup vs baseline: 1.0197x; 1.0197x over previous
"""MoE router (AutonomousRouter) for TRN2, 8 NeuronCores.

Computes reference:
    act    = einsum('bnd,edc->bnec', x, W)          B,N,D,E,C = 4,2048,2048,8,512
    logits = ||act||_2 over c                       [B,N,E]
    probs  = softmax(logits, -1)
    top-2 routing with capacity 640 (priority = order within k-major (choice, token) sequence)
    out    = stack([dispatch, combine])             [2,B,N,E,640] fp32

Sharding: data-parallel over tokens; core i <- tokens [i*1024, (i+1)*1024) of the
flattened [8192] token axis. Weights replicated.

Three device phases:
  A1 (coarse): single fp16 matmul per k-tile -> sum-of-squares ss [TOK, E].
      fp16 logit error is <~2e-3 while decision gaps are almost always larger;
      only tokens whose top1/2/3 logit gaps fall under GAP_T need exactness.
  A2 (exact):  fp16 hi/lo split (3 full-rate matmuls, fp32-grade: ~1e-7 logit
      err, measured on HW) for the <=NP ambiguous tokens, expert e on core e.
  B  (rows):   for each (token, choice) build the combine row
      prob*(iota==slot) densely as fp16 [2*TOK, cap]; host glue scatters rows
      into the zero output during unsharding (no indirect DMA) and derives
      dispatch = (combine != 0), exact since top-2 probs >> fp16 tiny.
Host glue between phases: softmax/top-2/capacity-cumsum on [8192, 8] scalars.
"""
import numpy as np

import concourse.bacc as bacc
import concourse.mybir as mybir
from concourse.tile import TileContext
from concourse.bass_utils import run_bass_kernel_spmd

P = 128          # partitions
B, N, D, E, C = 4, 2048, 2048, 8, 512
CAP = 640
NCORES = 8
T = B * N
TOK = T // NCORES           # tokens per core = 1024
NT = TOK // P               # token tiles per core = 8
KT = D // P                 # contraction tiles = 16

W_SCALE = 32.0              # keep fp16 weights away from subnormals
LO_SCALE = 4096.0           # 2^12 scaling for fp16 split low parts
GAP_T = 9e-3                # coarse logit-gap ambiguity threshold
NP_OPTS = (384, 512)        # padded ambiguous-token capacities (3 or 4 tiles)

f32 = mybir.dt.float32
f16 = mybir.dt.float16

_cache = {}
LAST_IN_MAPS_A1 = None   # kept for test harness re-runs/profiling
LAST_IN_MAPS_A2 = None
LAST_IN_MAPS_B = None
LAST_NP = None
LAST_NAMB = None


def _build_a1():
    """Coarse pass: ss[t, e] = sum_c (x[t] @ (32*w[e]))_c^2 in fp16 x fp16."""
    nc = bacc.Bacc("TRN2", target_bir_lowering=False, debug=False, num_devices=NCORES)
    xT = nc.dram_tensor("xT", [D, TOK], f16, kind="ExternalInput")
    w = nc.dram_tensor("w", [E, D, C], f16, kind="ExternalInput")
    ss_out = nc.dram_tensor("ss", [TOK, E], f32, kind="ExternalOutput")

    with TileContext(nc) as tc:
        with (
            tc.tile_pool(name="const", bufs=1) as cpool,
            tc.tile_pool(name="work", bufs=3) as spool,
            tc.tile_pool(name="psum", bufs=8, space="PSUM") as psum,
        ):
            # x^T and all of W live in SBUF (21 MB fp16). DMAs are issued in
            # consumption order; the first chunks are single k-blocks so the
            # first matmuls wait on ~0.8MB instead of 21MB.
            CHUNKS = [1, 3, 4, 4, 4]           # k-blocks per chunk, sums to KT
            CH0 = [sum(CHUNKS[:i]) for i in range(len(CHUNKS))]
            NCH = len(CHUNKS)

            def _x_chunk(q):
                nk = CHUNKS[q]
                t_ = cpool.tile([P, nk * TOK], f16, tag=f"xq{q}", name=f"x{q}")
                nc.sync.dma_start(
                    out=t_[:].rearrange("p (k n) -> p k n", k=nk),
                    in_=xT.ap()[CH0[q] * P:(CH0[q] + nk) * P, :]
                        .rearrange("(k p) n -> p k n", p=P),
                )
                return t_

            def _w_chunk(e, q):
                nk = CHUNKS[q]
                t_ = cpool.tile([P, nk * C], f16, tag=f"w{e}q{q}", name=f"w{e}_{q}")
                nc.sync.dma_start(
                    out=t_[:].rearrange("p (k c) -> p k c", k=nk),
                    in_=w.ap()[e, CH0[q] * P:(CH0[q] + nk) * P, :]
                        .rearrange("(k p) c -> p k c", p=P),
                )
                return t_

            xq, w_sb = [], {}
            for q in range(NCH):
                w_sb[(0, q)] = _w_chunk(0, q)
                xq.append(_x_chunk(q))
            for e in range(1, E):
                for q in range(NCH):
                    w_sb[(e, q)] = _w_chunk(e, q)

            ss_sb = cpool.tile([P, NT * E], f32, tag="ss")

            for e in range(E):
                for t in range(NT):
                    ps = psum.tile([P, C], f32, space="PSUM", tag="ps")
                    for k in range(KT):
                        q = max(i for i in range(NCH) if CH0[i] <= k)
                        kq = k - CH0[q]
                        nc.tensor.matmul(
                            ps[:],
                            lhsT=xq[q][:, kq * TOK + t * P: kq * TOK + (t + 1) * P],
                            rhs=w_sb[(e, q)][:, kq * C:(kq + 1) * C],
                            start=(k == 0), stop=(k == KT - 1),
                        )
                    sq = spool.tile([P, C], f32, tag="sq")
                    nc.scalar.activation(sq[:], ps[:], mybir.ActivationFunctionType.Square,
                                         accum_out=ss_sb[:, t * E + e: t * E + e + 1])
            nc.sync.dma_start(
                out=ss_out.ap()[:, :].rearrange("(t p) e -> p t e", p=P),
                in_=ss_sb[:].rearrange("p (t e) -> p t e", e=E))
    nc.compile()
    return nc


def _build_a2(npad):
    """Exact pass: fp32-grade sumsq for npad gathered tokens x one expert/core.

    x = xh + xls/LO_SCALE, w = wh + wls/LO_SCALE (all fp16);
    a ~= xh@wh + (xh@wls + xls@wh)/LO_SCALE  (xl*wl term ~2^-22 rel, dropped).
    """
    nc = bacc.Bacc("TRN2", target_bir_lowering=False, debug=False, num_devices=NCORES)
    xh = nc.dram_tensor("xh", [D, npad], f16, kind="ExternalInput")
    xls = nc.dram_tensor("xls", [D, npad], f16, kind="ExternalInput")
    wh = nc.dram_tensor("wh", [D, C], f16, kind="ExternalInput")
    wls = nc.dram_tensor("wls", [D, C], f16, kind="ExternalInput")
    ss_out = nc.dram_tensor("ss", [npad, 1], f32, kind="ExternalOutput")
    NT2 = npad // P

    CHUNKS = [1, 3, 4, 4, 4]     # k-blocks per chunk, consumption order
    CH0 = [sum(CHUNKS[:i]) for i in range(len(CHUNKS))]
    NCH = len(CHUNKS)

    with TileContext(nc) as tc:
        with (
            tc.tile_pool(name="const", bufs=1) as cpool,
            tc.tile_pool(name="work", bufs=3) as spool,
            tc.tile_pool(name="psum", bufs=1, space="PSUM") as psum,
        ):
            def _chunk(dram, cols, q, tag):
                nk = CHUNKS[q]
                t_ = cpool.tile([P, nk * cols], f16, tag=tag, name=tag)
                nc.sync.dma_start(
                    out=t_[:].rearrange("p (k n) -> p k n", k=nk),
                    in_=dram.ap()[CH0[q] * P:(CH0[q] + nk) * P, :]
                        .rearrange("(k p) n -> p k n", p=P),
                )
                return t_

            wh_q, xh_q, wls_q, xls_q = [], [], [], []
            for q in range(NCH):
                wh_q.append(_chunk(wh, C, q, f"wh{q}"))
                xh_q.append(_chunk(xh, npad, q, f"xh{q}"))
                wls_q.append(_chunk(wls, C, q, f"wls{q}"))
                xls_q.append(_chunk(xls, npad, q, f"xls{q}"))
            ss_sb = cpool.tile([P, NT2], f32, tag="ss")

            # k-outer / tile-inner: up to 8 PSUM banks = NT2 tiles x {hh, cross};
            # each DMA chunk is consumed by all tiles before the next chunk.
            # The last chunk runs tile-outer so tiles finish staggered and the
            # combine/Square tail overlaps the matmul stream.
            ps_hh = [psum.tile([P, C], f32, space="PSUM", tag=f"ph{t}",
                               name=f"ph{t}") for t in range(NT2)]
            ps_cr = [psum.tile([P, C], f32, space="PSUM", tag=f"pc{t}",
                               name=f"pc{t}") for t in range(NT2)]

            def _mms(t, k):
                q = max(i for i in range(NCH) if CH0[i] <= k)
                kq = k - CH0[q]
                xh_blk = xh_q[q][:, kq * npad + t * P: kq * npad + (t + 1) * P]
                xls_blk = xls_q[q][:, kq * npad + t * P: kq * npad + (t + 1) * P]
                wh_blk = wh_q[q][:, kq * C:(kq + 1) * C]
                wls_blk = wls_q[q][:, kq * C:(kq + 1) * C]
                nc.tensor.matmul(ps_hh[t][:], lhsT=xh_blk, rhs=wh_blk,
                                 start=(k == 0), stop=(k == KT - 1))
                nc.tensor.matmul(ps_cr[t][:], lhsT=xh_blk, rhs=wls_blk,
                                 start=(k == 0), stop=False)
                nc.tensor.matmul(ps_cr[t][:], lhsT=xls_blk, rhs=wh_blk,
                                 start=False, stop=(k == KT - 1))

            def _finish(t):
                a = spool.tile([P, C], f32, tag="a")
                nc.vector.tensor_scalar(a[:], ps_cr[t][:], 1.0 / LO_SCALE, None,
                                        op0=mybir.AluOpType.mult)
                nc.vector.tensor_add(out=a[:], in0=a[:], in1=ps_hh[t][:])
                sq = spool.tile([P, C], f32, tag="sq")
                nc.scalar.activation(sq[:], a[:], mybir.ActivationFunctionType.Square,
                                     accum_out=ss_sb[:, t:t + 1])

            LASTQ = CH0[-1]                     # k-start of the last chunk
            for k in range(LASTQ):
                for t in range(NT2):
                    _mms(t, k)
            for t in range(NT2):
                for k in range(LASTQ, KT):
                    _mms(t, k)
                _finish(t)
            nc.sync.dma_start(
                out=ss_out.ap()[:, 0].rearrange("(t p) -> p t", p=P), in_=ss_sb[:])
    nc.compile()
    return nc


def _build_b(cap=CAP):
    """Row builder: per (token, choice) the combine row prob*(iota==slot),
    dense fp16 [2*TOK, cap]. Host glue scatters rows by (token, chosen
    expert) while unsharding and derives dispatch = (combine != 0), exact
    because top-2 softmax probs (>~1e-2) never round to fp16 zero."""
    NR = 2 * TOK
    NG = NR // P
    nc = bacc.Bacc("TRN2", target_bir_lowering=False, debug=False, num_devices=NCORES)
    slot = nc.dram_tensor("slot", [NR, 1], f32, kind="ExternalInput")
    prob = nc.dram_tensor("prob", [NR, 1], f32, kind="ExternalInput")
    iota_cap = nc.dram_tensor("iota_cap", [P, cap], f16, kind="ExternalInput")
    rows = nc.dram_tensor("rows", [NR, cap], f16, kind="ExternalOutput")

    with TileContext(nc) as tc:
        with (
            tc.tile_pool(name="const", bufs=1) as cpool,
            tc.tile_pool(name="work", bufs=4) as spool,
        ):
            iota_sb = cpool.tile([P, cap], f16, tag="iota")
            nc.sync.dma_start(out=iota_sb[:], in_=iota_cap.ap()[:, :])
            sl = cpool.tile([P, NG], f32, tag="sl")
            nc.sync.dma_start(out=sl[:], in_=slot.ap()[:, 0].rearrange("(g p) -> p g", p=P))
            pr = cpool.tile([P, NG], f32, tag="pr")
            nc.sync.dma_start(out=pr[:], in_=prob.ap()[:, 0].rearrange("(g p) -> p g", p=P))
            GPB = 8                           # groups batched per DMA
            for g0 in range(0, NG, GPB):
                rtile = spool.tile([P, GPB * cap], f16, tag="rt")
                for j in range(GPB):
                    g = g0 + j
                    nc.vector.tensor_scalar(rtile[:, j * cap:(j + 1) * cap],
                                            iota_sb[:], sl[:, g:g + 1],
                                            pr[:, g:g + 1],
                                            op0=mybir.AluOpType.is_equal,
                                            op1=mybir.AluOpType.mult)
                nc.sync.dma_start(
                    out=rows.ap()[g0 * P:(g0 + GPB) * P, :]
                        .rearrange("(g p) c -> p g c", p=P),
                    in_=rtile[:].rearrange("p (g c) -> p g c", g=GPB))
    nc.compile()
    return nc


def _get(name, builder):
    if name not in _cache:
        _cache[name] = builder()
    return _cache[name]


def kernel(token_inputs, bottleneck_weights, expert_capacity):
    global LAST_IN_MAPS_A1, LAST_IN_MAPS_A2, LAST_IN_MAPS_B, LAST_NP, LAST_NAMB
    x = np.ascontiguousarray(np.asarray(token_inputs, dtype=np.float32)).reshape(T, D)
    w = np.ascontiguousarray(np.asarray(bottleneck_weights, dtype=np.float32))
    cap = int(expert_capacity)
    assert 0 < cap <= 2048   # iota/slot compares rely on exact fp16 integers
    core_ids = list(range(NCORES))

    # ---- phase A1: coarse fp16 sum-of-squares ----
    w16 = np.ascontiguousarray((w * W_SCALE).astype(np.float16))
    in_maps_a1 = []
    for c in core_ids:
        xT = np.ascontiguousarray(x[c * TOK:(c + 1) * TOK].T).astype(np.float16)
        in_maps_a1.append({"xT": xT, "w": w16})
    LAST_IN_MAPS_A1 = in_maps_a1
    nc1 = _get("a1", _build_a1)
    res1 = run_bass_kernel_spmd(nc1, in_maps_a1, core_ids)
    ss = np.concatenate([r["ss"] for r in res1.results], 0).astype(np.float64)
    ss /= W_SCALE * W_SCALE
    L = np.sqrt(ss)                                   # coarse logits [T, E]

    # ---- ambiguous tokens: any of the top1/2/3 coarse gaps under GAP_T ----
    l_sorted = np.sort(L, axis=1)[:, ::-1]
    rel_gap = np.minimum(l_sorted[:, 0] - l_sorted[:, 1],
                         l_sorted[:, 1] - l_sorted[:, 2])
    amb = np.flatnonzero(rel_gap < GAP_T)
    namb = len(amb)
    NP = next((n for n in NP_OPTS if namb <= n), None)
    assert NP is not None, f"ambiguous token overflow: {namb} > {NP_OPTS[-1]}"
    LAST_NP, LAST_NAMB = NP, namb

    # ---- phase A2: exact sumsq for ambiguous tokens (expert e on core e) ----
    xaT = np.zeros((D, NP), np.float32)
    xaT[:, :namb] = x[amb].T
    xh = xaT.astype(np.float16)
    xls = ((xaT - xh.astype(np.float32)) * LO_SCALE).astype(np.float16)
    in_maps_a2 = []
    for e in range(NCORES):
        we = np.ascontiguousarray(w[e])
        wh = we.astype(np.float16)
        wls = ((we - wh.astype(np.float32)) * LO_SCALE).astype(np.float16)
        in_maps_a2.append({"xh": xh, "xls": xls, "wh": wh, "wls": wls})
    LAST_IN_MAPS_A2 = in_maps_a2
    nc2 = _get(f"a2_{NP}", lambda: _build_a2(NP))
    res2 = run_bass_kernel_spmd(nc2, in_maps_a2, core_ids)
    if namb:
        ss_ex = np.stack([res2.results[e]["ss"].reshape(-1)[:namb]
                          for e in range(NCORES)], 1).astype(np.float64)
        L[amb] = np.sqrt(ss_ex)

    # ---- host glue: top-2 (stable => lower index on ties, like lax.top_k),
    # softmax probs, capacity priorities over the k-major (choice, token) seq.
    order = np.argsort(-L, axis=1, kind="stable")
    e0, e1 = order[:, 0], order[:, 1]
    m = L.max(1, keepdims=True)
    pexp = np.exp(L - m)
    probs = pexp / pexp.sum(1, keepdims=True)
    slot = np.empty((T, 2), np.int64)
    for b in range(B):
        bsl = slice(b * N, (b + 1) * N)
        seq = np.concatenate([e0[bsl], e1[bsl]])
        onehot = seq[:, None] == np.arange(E)[None, :]
        pri = onehot.cumsum(0) - 1
        pv = pri[np.arange(2 * N), seq]
        slot[bsl, 0] = pv[:N]
        slot[bsl, 1] = pv[N:]

    # ---- phase B: build rows on device ----
    ar = np.arange(T)
    p0 = probs[ar, e0].astype(np.float32)
    p1 = probs[ar, e1].astype(np.float32)
    iota16 = np.tile(np.arange(cap, dtype=np.float16), (P, 1))
    in_maps_b = []
    for c in core_ids:
        tsl = slice(c * TOK, (c + 1) * TOK)
        in_maps_b.append({
            "slot": np.concatenate([slot[tsl, 0], slot[tsl, 1]])
                      .astype(np.float32)[:, None],
            "prob": np.concatenate([p0[tsl], p1[tsl]]).astype(np.float32)[:, None],
            "iota_cap": iota16,
        })
    LAST_IN_MAPS_B = in_maps_b
    nc3 = _get(f"b{cap}", lambda: _build_b(cap))
    res3 = run_bass_kernel_spmd(nc3, in_maps_b, core_ids)

    # ---- unshard: scatter rows into the dense output ----
    out = np.zeros((2, T, E, cap), np.float32)
    for c in core_ids:
        rows = res3.results[c]["rows"]                  # [2*TOK, cap] f16
        toks = np.arange(c * TOK, (c + 1) * TOK)
        for k, ek in ((0, e0), (1, e1)):
            rk = rows[k * TOK:(k + 1) * TOK].astype(np.float32)
            out[0, toks, ek[toks]] = (rk != 0.0).astype(np.float32)
            out[1, toks, ek[toks]] = rk
    return out.reshape(2, B, N, E, cap)


# revision 25
# speedup vs baseline: 1.0437x; 1.0235x over previous
"""MoE router (AutonomousRouter) for TRN2, 8 NeuronCores.

Computes reference:
    act    = einsum('bnd,edc->bnec', x, W)          B,N,D,E,C = 4,2048,2048,8,512
    logits = ||act||_2 over c                       [B,N,E]
    probs  = softmax(logits, -1)
    top-2 routing with capacity 640 (priority = order within k-major (choice, token) sequence)
    out    = stack([dispatch, combine])             [2,B,N,E,640] fp32

Sharding: data-parallel over tokens; core i <- tokens [i*1024, (i+1)*1024) of the
flattened [8192] token axis. Weights replicated.

Three device phases:
  A1 (coarse): single fp16 matmul per k-tile -> sum-of-squares ss [TOK, E].
      fp16 logit error is <~2e-3 while decision gaps are almost always larger;
      only tokens whose top1/2/3 logit gaps fall under GAP_T need exactness.
  A2 (exact):  fp16 hi/lo split (3 full-rate matmuls, fp32-grade: ~1e-7 logit
      err, measured on HW) for the <=NP ambiguous tokens, expert e on core e.
  B  (rows):   for each (token, choice) build the combine row
      prob*(iota==slot) densely as fp16 [2*TOK, cap]; host glue scatters rows
      into the zero output during unsharding (no indirect DMA) and derives
      dispatch = (combine != 0), exact since top-2 probs >> fp16 tiny.
Host glue between phases: softmax/top-2/capacity-cumsum on [8192, 8] scalars.
"""
import numpy as np

import concourse.bacc as bacc
import concourse.mybir as mybir
from concourse.tile import TileContext
from concourse.bass_utils import run_bass_kernel_spmd

P = 128          # partitions
B, N, D, E, C = 4, 2048, 2048, 8, 512
CAP = 640
NCORES = 8
T = B * N
TOK = T // NCORES           # tokens per core = 1024
NT = TOK // P               # token tiles per core = 8
KT = D // P                 # contraction tiles = 16

W_SCALE = 32.0              # keep fp16 weights away from subnormals
LO_SCALE = 4096.0           # 2^12 scaling for fp16 split low parts
GAP_T = 5e-3                # coarse logit-gap ambiguity threshold; device coarse
                            # err max is 1.6e-3, so non-ambiguous orderings hold
                            # with >1.3x margin over the 2*err flip bound
NP_OPTS = (256, 384, 512)   # padded ambiguous-token capacities (2/3/4 tiles)

f32 = mybir.dt.float32
f16 = mybir.dt.float16

_cache = {}
LAST_IN_MAPS_A1 = None   # kept for test harness re-runs/profiling
LAST_IN_MAPS_A2 = None
LAST_IN_MAPS_B = None
LAST_NP = None
LAST_NAMB = None


def _build_a1():
    """Coarse pass: ss[t, e] = sum_c (x[t] @ (32*w[e]))_c^2 in fp16 x fp16."""
    nc = bacc.Bacc("TRN2", target_bir_lowering=False, debug=False, num_devices=NCORES)
    xT = nc.dram_tensor("xT", [D, TOK], f16, kind="ExternalInput")
    w = nc.dram_tensor("w", [E, D, C], f16, kind="ExternalInput")
    ss_out = nc.dram_tensor("ss", [TOK, E], f32, kind="ExternalOutput")

    with TileContext(nc) as tc:
        with (
            tc.tile_pool(name="const", bufs=1) as cpool,
            tc.tile_pool(name="work", bufs=3) as spool,
            tc.tile_pool(name="psum", bufs=8, space="PSUM") as psum,
        ):
            # x^T and all of W live in SBUF (21 MB fp16). DMAs are issued in
            # consumption order; the first chunks are single k-blocks so the
            # first matmuls wait on ~0.8MB instead of 21MB.
            CHUNKS = [1, 3, 4, 4, 4]           # k-blocks per chunk, sums to KT
            CH0 = [sum(CHUNKS[:i]) for i in range(len(CHUNKS))]
            NCH = len(CHUNKS)

            def _x_chunk(q):
                nk = CHUNKS[q]
                t_ = cpool.tile([P, nk * TOK], f16, tag=f"xq{q}", name=f"x{q}")
                nc.sync.dma_start(
                    out=t_[:].rearrange("p (k n) -> p k n", k=nk),
                    in_=xT.ap()[CH0[q] * P:(CH0[q] + nk) * P, :]
                        .rearrange("(k p) n -> p k n", p=P),
                )
                return t_

            def _w_chunk(e, q):
                nk = CHUNKS[q]
                t_ = cpool.tile([P, nk * C], f16, tag=f"w{e}q{q}", name=f"w{e}_{q}")
                nc.sync.dma_start(
                    out=t_[:].rearrange("p (k c) -> p k c", k=nk),
                    in_=w.ap()[e, CH0[q] * P:(CH0[q] + nk) * P, :]
                        .rearrange("(k p) c -> p k c", p=P),
                )
                return t_

            xq, w_sb = [], {}
            for q in range(NCH):
                w_sb[(0, q)] = _w_chunk(0, q)
                xq.append(_x_chunk(q))
            for e in range(1, E):
                for q in range(NCH):
                    w_sb[(e, q)] = _w_chunk(e, q)

            ss_sb = cpool.tile([P, NT * E], f32, tag="ss")

            for e in range(E):
                for t in range(NT):
                    ps = psum.tile([P, C], f32, space="PSUM", tag="ps")
                    for k in range(KT):
                        q = max(i for i in range(NCH) if CH0[i] <= k)
                        kq = k - CH0[q]
                        nc.tensor.matmul(
                            ps[:],
                            lhsT=xq[q][:, kq * TOK + t * P: kq * TOK + (t + 1) * P],
                            rhs=w_sb[(e, q)][:, kq * C:(kq + 1) * C],
                            start=(k == 0), stop=(k == KT - 1),
                        )
                    sq = spool.tile([P, C], f32, tag="sq")
                    nc.scalar.activation(sq[:], ps[:], mybir.ActivationFunctionType.Square,
                                         accum_out=ss_sb[:, t * E + e: t * E + e + 1])
            nc.sync.dma_start(
                out=ss_out.ap()[:, :].rearrange("(t p) e -> p t e", p=P),
                in_=ss_sb[:].rearrange("p (t e) -> p t e", e=E))
    nc.compile()
    return nc


def _build_a2(npad):
    """Exact pass: fp32-grade sumsq for npad gathered tokens x one expert/core.

    x = xh + xls/LO_SCALE, w = wh + wls/LO_SCALE (all fp16);
    a ~= xh@wh + (xh@wls + xls@wh)/LO_SCALE  (xl*wl term ~2^-22 rel, dropped).
    """
    nc = bacc.Bacc("TRN2", target_bir_lowering=False, debug=False, num_devices=NCORES)
    xh = nc.dram_tensor("xh", [D, npad], f16, kind="ExternalInput")
    xls = nc.dram_tensor("xls", [D, npad], f16, kind="ExternalInput")
    wh = nc.dram_tensor("wh", [D, C], f16, kind="ExternalInput")
    wls = nc.dram_tensor("wls", [D, C], f16, kind="ExternalInput")
    ss_out = nc.dram_tensor("ss", [npad, 1], f32, kind="ExternalOutput")
    NT2 = npad // P

    CHUNKS = [1, 3, 4, 4, 4]     # k-blocks per chunk, consumption order
    CH0 = [sum(CHUNKS[:i]) for i in range(len(CHUNKS))]
    NCH = len(CHUNKS)

    with TileContext(nc) as tc:
        with (
            tc.tile_pool(name="const", bufs=1) as cpool,
            tc.tile_pool(name="work", bufs=3) as spool,
            tc.tile_pool(name="psum", bufs=1, space="PSUM") as psum,
        ):
            def _chunk(dram, cols, q, tag):
                nk = CHUNKS[q]
                t_ = cpool.tile([P, nk * cols], f16, tag=tag, name=tag)
                nc.sync.dma_start(
                    out=t_[:].rearrange("p (k n) -> p k n", k=nk),
                    in_=dram.ap()[CH0[q] * P:(CH0[q] + nk) * P, :]
                        .rearrange("(k p) n -> p k n", p=P),
                )
                return t_

            wh_q, xh_q, wls_q, xls_q = [], [], [], []
            for q in range(NCH):
                wh_q.append(_chunk(wh, C, q, f"wh{q}"))
                xh_q.append(_chunk(xh, npad, q, f"xh{q}"))
                wls_q.append(_chunk(wls, C, q, f"wls{q}"))
                xls_q.append(_chunk(xls, npad, q, f"xls{q}"))
            ss_sb = cpool.tile([P, NT2], f32, tag="ss")

            # k-outer / tile-inner: up to 8 PSUM banks = NT2 tiles x {hh, cross};
            # each DMA chunk is consumed by all tiles before the next chunk.
            # The last chunk runs tile-outer so tiles finish staggered and the
            # combine/Square tail overlaps the matmul stream.
            ps_hh = [psum.tile([P, C], f32, space="PSUM", tag=f"ph{t}",
                               name=f"ph{t}") for t in range(NT2)]
            ps_cr = [psum.tile([P, C], f32, space="PSUM", tag=f"pc{t}",
                               name=f"pc{t}") for t in range(NT2)]

            def _mms(t, k):
                q = max(i for i in range(NCH) if CH0[i] <= k)
                kq = k - CH0[q]
                xh_blk = xh_q[q][:, kq * npad + t * P: kq * npad + (t + 1) * P]
                xls_blk = xls_q[q][:, kq * npad + t * P: kq * npad + (t + 1) * P]
                wh_blk = wh_q[q][:, kq * C:(kq + 1) * C]
                wls_blk = wls_q[q][:, kq * C:(kq + 1) * C]
                nc.tensor.matmul(ps_hh[t][:], lhsT=xh_blk, rhs=wh_blk,
                                 start=(k == 0), stop=(k == KT - 1))
                nc.tensor.matmul(ps_cr[t][:], lhsT=xh_blk, rhs=wls_blk,
                                 start=(k == 0), stop=False)
                nc.tensor.matmul(ps_cr[t][:], lhsT=xls_blk, rhs=wh_blk,
                                 start=False, stop=(k == KT - 1))

            def _finish(t):
                a = spool.tile([P, C], f32, tag="a")
                nc.vector.tensor_scalar(a[:], ps_cr[t][:], 1.0 / LO_SCALE, None,
                                        op0=mybir.AluOpType.mult)
                nc.vector.tensor_add(out=a[:], in0=a[:], in1=ps_hh[t][:])
                sq = spool.tile([P, C], f32, tag="sq")
                nc.scalar.activation(sq[:], a[:], mybir.ActivationFunctionType.Square,
                                     accum_out=ss_sb[:, t:t + 1])

            LASTQ = CH0[-1]                     # k-start of the last chunk
            for k in range(LASTQ):
                for t in range(NT2):
                    _mms(t, k)
            for t in range(NT2):
                for k in range(LASTQ, KT):
                    _mms(t, k)
                _finish(t)
            nc.sync.dma_start(
                out=ss_out.ap()[:, 0].rearrange("(t p) -> p t", p=P), in_=ss_sb[:])
    nc.compile()
    return nc


def _build_b(cap=CAP):
    """Row builder: per (token, choice) the combine row prob*(iota==slot),
    dense fp16 [2*TOK, cap]. Host glue scatters rows by (token, chosen
    expert) while unsharding and derives dispatch = (combine != 0), exact
    because top-2 softmax probs (>~1e-2) never round to fp16 zero."""
    NR = 2 * TOK
    NG = NR // P
    nc = bacc.Bacc("TRN2", target_bir_lowering=False, debug=False, num_devices=NCORES)
    slot = nc.dram_tensor("slot", [NR, 1], f32, kind="ExternalInput")
    prob = nc.dram_tensor("prob", [NR, 1], f32, kind="ExternalInput")
    iota_cap = nc.dram_tensor("iota_cap", [P, cap], f16, kind="ExternalInput")
    rows = nc.dram_tensor("rows", [NR, cap], f16, kind="ExternalOutput")

    with TileContext(nc) as tc:
        with (
            tc.tile_pool(name="const", bufs=1) as cpool,
            tc.tile_pool(name="work", bufs=4) as spool,
        ):
            iota_sb = cpool.tile([P, cap], f16, tag="iota")
            nc.sync.dma_start(out=iota_sb[:], in_=iota_cap.ap()[:, :])
            sl = cpool.tile([P, NG], f32, tag="sl")
            nc.sync.dma_start(out=sl[:], in_=slot.ap()[:, 0].rearrange("(g p) -> p g", p=P))
            pr = cpool.tile([P, NG], f32, tag="pr")
            nc.sync.dma_start(out=pr[:], in_=prob.ap()[:, 0].rearrange("(g p) -> p g", p=P))
            GPB = 8                           # groups batched per DMA
            for g0 in range(0, NG, GPB):
                rtile = spool.tile([P, GPB * cap], f16, tag="rt")
                for j in range(GPB):
                    g = g0 + j
                    nc.vector.tensor_scalar(rtile[:, j * cap:(j + 1) * cap],
                                            iota_sb[:], sl[:, g:g + 1],
                                            pr[:, g:g + 1],
                                            op0=mybir.AluOpType.is_equal,
                                            op1=mybir.AluOpType.mult)
                nc.sync.dma_start(
                    out=rows.ap()[g0 * P:(g0 + GPB) * P, :]
                        .rearrange("(g p) c -> p g c", p=P),
                    in_=rtile[:].rearrange("p (g c) -> p g c", g=GPB))
    nc.compile()
    return nc


def _get(name, builder):
    if name not in _cache:
        _cache[name] = builder()
    return _cache[name]


def kernel(token_inputs, bottleneck_weights, expert_capacity):
    global LAST_IN_MAPS_A1, LAST_IN_MAPS_A2, LAST_IN_MAPS_B, LAST_NP, LAST_NAMB
    x = np.ascontiguousarray(np.asarray(token_inputs, dtype=np.float32)).reshape(T, D)
    w = np.ascontiguousarray(np.asarray(bottleneck_weights, dtype=np.float32))
    cap = int(expert_capacity)
    assert 0 < cap <= 2048   # iota/slot compares rely on exact fp16 integers
    core_ids = list(range(NCORES))

    # ---- phase A1: coarse fp16 sum-of-squares ----
    w16 = np.ascontiguousarray((w * W_SCALE).astype(np.float16))
    in_maps_a1 = []
    for c in core_ids:
        xT = np.ascontiguousarray(x[c * TOK:(c + 1) * TOK].T).astype(np.float16)
        in_maps_a1.append({"xT": xT, "w": w16})
    LAST_IN_MAPS_A1 = in_maps_a1
    nc1 = _get("a1", _build_a1)
    res1 = run_bass_kernel_spmd(nc1, in_maps_a1, core_ids)
    ss = np.concatenate([r["ss"] for r in res1.results], 0).astype(np.float64)
    ss /= W_SCALE * W_SCALE
    L = np.sqrt(ss)                                   # coarse logits [T, E]

    # ---- ambiguous tokens: any of the top1/2/3 coarse gaps under GAP_T ----
    l_sorted = np.sort(L, axis=1)[:, ::-1]
    rel_gap = np.minimum(l_sorted[:, 0] - l_sorted[:, 1],
                         l_sorted[:, 1] - l_sorted[:, 2])
    amb = np.flatnonzero(rel_gap < GAP_T)
    namb = len(amb)
    NP = next((n for n in NP_OPTS if namb <= n), None)
    assert NP is not None, f"ambiguous token overflow: {namb} > {NP_OPTS[-1]}"
    LAST_NP, LAST_NAMB = NP, namb

    # ---- phase A2: exact sumsq for ambiguous tokens (expert e on core e) ----
    xaT = np.zeros((D, NP), np.float32)
    xaT[:, :namb] = x[amb].T
    xh = xaT.astype(np.float16)
    xls = ((xaT - xh.astype(np.float32)) * LO_SCALE).astype(np.float16)
    in_maps_a2 = []
    for e in range(NCORES):
        we = np.ascontiguousarray(w[e])
        wh = we.astype(np.float16)
        wls = ((we - wh.astype(np.float32)) * LO_SCALE).astype(np.float16)
        in_maps_a2.append({"xh": xh, "xls": xls, "wh": wh, "wls": wls})
    LAST_IN_MAPS_A2 = in_maps_a2
    nc2 = _get(f"a2_{NP}", lambda: _build_a2(NP))
    res2 = run_bass_kernel_spmd(nc2, in_maps_a2, core_ids)
    if namb:
        ss_ex = np.stack([res2.results[e]["ss"].reshape(-1)[:namb]
                          for e in range(NCORES)], 1).astype(np.float64)
        L[amb] = np.sqrt(ss_ex)

    # ---- host glue: top-2 (stable => lower index on ties, like lax.top_k),
    # softmax probs, capacity priorities over the k-major (choice, token) seq.
    order = np.argsort(-L, axis=1, kind="stable")
    e0, e1 = order[:, 0], order[:, 1]
    m = L.max(1, keepdims=True)
    pexp = np.exp(L - m)
    probs = pexp / pexp.sum(1, keepdims=True)
    slot = np.empty((T, 2), np.int64)
    for b in range(B):
        bsl = slice(b * N, (b + 1) * N)
        seq = np.concatenate([e0[bsl], e1[bsl]])
        onehot = seq[:, None] == np.arange(E)[None, :]
        pri = onehot.cumsum(0) - 1
        pv = pri[np.arange(2 * N), seq]
        slot[bsl, 0] = pv[:N]
        slot[bsl, 1] = pv[N:]

    # ---- phase B: build rows on device ----
    ar = np.arange(T)
    p0 = probs[ar, e0].astype(np.float32)
    p1 = probs[ar, e1].astype(np.float32)
    iota16 = np.tile(np.arange(cap, dtype=np.float16), (P, 1))
    in_maps_b = []
    for c in core_ids:
        tsl = slice(c * TOK, (c + 1) * TOK)
        in_maps_b.append({
            "slot": np.concatenate([slot[tsl, 0], slot[tsl, 1]])
                      .astype(np.float32)[:, None],
            "prob": np.concatenate([p0[tsl], p1[tsl]]).astype(np.float32)[:, None],
            "iota_cap": iota16,
        })
    LAST_IN_MAPS_B = in_maps_b
    nc3 = _get(f"b{cap}", lambda: _build_b(cap))
    res3 = run_bass_kernel_spmd(nc3, in_maps_b, core_ids)

    # ---- unshard: scatter rows into the dense output ----
    out = np.zeros((2, T, E, cap), np.float32)
    for c in core_ids:
        rows = res3.results[c]["rows"]                  # [2*TOK, cap] f16
        toks = np.arange(c * TOK, (c + 1) * TOK)
        for k, ek in ((0, e0), (1, e1)):
            rk = rows[k * TOK:(k + 1) * TOK].astype(np.float32)
            out[0, toks, ek[toks]] = (rk != 0.0).astype(np.float32)
            out[1, toks, ek[toks]] = rk
    return out.reshape(2, B, N, E, cap)
